# revision 28
# baseline (speedup 1.0000x reference)
"""Trainium2 Bass kernel for an attention-LSTM decoder (Bahdanau attention +
LSTM cell + generator head), data-parallel over 8 NeuronCores.

Shapes (hardcoded): B=1024, T=64, D=256, H=256, C=96, steps S=26.
Each core handles 128 batch rows.

Device layouts (per core, b = 128) — everything in the score/context chain
is stored T-MAJOR so every DVE op is stride-1-inner (2x mode) and the
context-reduce matmuls read contiguous moving slices:
  - h_tt2  [d' 128, t, k, b]   (H^T, t-major)
  - projT2 [h' 128, k, t, b]   (batch_H @ Wi2h)^T
  - z/th/ar/tmp per t-quarter, [p, k, t', b] / [p, t', k, b]
  - w_score replicated across 128 PE columns -> e (and exp(e)) come out of
    the e-matmul replicated on all partitions; context = h_tt2 * alphaR on
    DVE, reduced over t by PE identity-matmuls (contiguous moving slices,
    2-way PSUM interleave), already transposed [d', b] for the gates matmul.
  - Z = sum_t exp(e): gpsimd folds t' 16->8 per quarter; DVE folds the rest.
  - LSTM/gates run natural [b, 4H]; h transposed back via the DMA XBAR.
Matmul operands fp16, PSUM accumulation fp32.

Host-side prep (numpy): fp16 casts, batch_H transpose, one-hot text,
b_lstm folded into the one-hot weight rows, bg added on the host at the end.
"""

import sys

for _p in ("/opt/trn_rl_repo", "/root/.axon_site/_ro/trn_rl_repo"):
    if _p not in sys.path:
        sys.path.insert(0, _p)

import numpy as np

import concourse.bass as bass
import concourse.tile as tile
from concourse import mybir
from concourse.bass_utils import run_bass_kernel_spmd
from concourse.masks import make_identity

dt = mybir.dt
AF = mybir.ActivationFunctionType
ALU = mybir.AluOpType

NCORES = 8
B, T, D, H, C = 1024, 64, 256, 256, 96
S = 26  # num steps = batch_max_length + 1
BC = B // NCORES  # 128 batch rows per core
KT = 2  # 256 = 2 x 128 tiles for d/h contraction
TB = BC * T  # 8192 positions per step
NCHUNK = 512  # psum-bank-limited matmul N
NQ = 4  # t-quarters per step
TQ = T // NQ  # 16 t columns per quarter
NH = 8  # half-quarters
TH = T // NH  # 8 t columns per half-quarter

_CACHE = {}


def _split_excess_waits(nc, max_waits=1):
    """This container's walrus rejects instructions carrying more than
    ~max_waits semaphore waits ("Too many sync wait commands"). Hoist excess
    waits onto InstNoOp instructions inserted just before, on the same engine
    (per-engine program order makes this semantics-preserving)."""
    nid = [0]
    for f in nc.m.functions:
        for blk in f.blocks:
            insts = blk.instructions
            out = []
            changed = False
            for ins in insts:
                si = ins.sync_info
                ow = list(si.on_wait) if si is not None and si.on_wait else []
                if len(ow) > max_waits:
                    changed = True
                    while len(ow) > max_waits:
                        take, ow = ow[:max_waits], ow[max_waits:]
                        nid[0] += 1
                        nop = mybir.InstNoOp(
                            name=f"WSPLIT-{nid[0]}", engine=ins.engine,
                            sync_info=mybir.SyncInfo(on_wait=take,
                                                     on_update=[]))
                        nc.register_instruction(nop, overwrite=True)
                        out.append(nop)
                    ins.sync_info = mybir.SyncInfo(
                        on_wait=ow, on_update=list(si.on_update or []))
                out.append(ins)
            if changed:
                blk.instructions = out


def _build():
    nc = bass.Bass("TRN2", target_bir_lowering=False)
    f16, f32 = dt.float16, dt.float32

    h_t_d = nc.declare_dram_parameter("h_t", [D, T, BC], f16, isOutput=False)
    oneh_d = nc.declare_dram_parameter("onehot", [C, S, BC], f16, isOutput=False)
    wi2h_d = nc.declare_dram_parameter("wi2h", [D, H], f16, isOutput=False)
    wh2h_d = nc.declare_dram_parameter("wh2h", [H, H], f16, isOutput=False)
    bh2h_d = nc.declare_dram_parameter("bh2hT", [H, 1], f32, isOutput=False)
    wsc_d = nc.declare_dram_parameter("wsc", [H, 128], f16, isOutput=False)
    wxc_d = nc.declare_dram_parameter("wxc", [D, 4 * H], f16, isOutput=False)
    wxo_d = nc.declare_dram_parameter("wxo", [C, 4 * H], f16, isOutput=False)
    wh_d = nc.declare_dram_parameter("wh", [H, 4 * H], f16, isOutput=False)
    wg_d = nc.declare_dram_parameter("wg", [H, C], f16, isOutput=False)
    probs_d = nc.declare_dram_parameter("probsT", [C, S, BC], f32, isOutput=True)

    with tile.TileContext(nc) as tc:
        import contextlib
        ctx = contextlib.ExitStack()
        with ctx:
            singles = ctx.enter_context(tc.tile_pool(name="singles", bufs=1))
            psE = ctx.enter_context(tc.tile_pool(name="psE", bufs=2, space="PSUM"))
            psG = ctx.enter_context(tc.tile_pool(name="psG", bufs=1, space="PSUM"))
            psM = ctx.enter_context(tc.tile_pool(name="psM", bufs=2, space="PSUM"))

            # ---- persistent SBUF state ----
            h_tt2 = singles.tile([128, T, KT, BC], f16)  # H^T t-major
            projT2 = singles.tile([128, KT, T, BC], f16)
            hidT = singles.tile([128, KT, S + 1, BC], f16)  # h states; slot 0 = 0
            oneh = singles.tile([C, S, BC], f16)
            wi2h = singles.tile([128, KT, H], f16)
            wh2h = singles.tile([128, KT, H], f16)
            bh2hT = singles.tile([128, KT, 1], f32)
            wsc = singles.tile([128, KT, 128], f16)
            wxc = singles.tile([128, KT, 4 * H], f16)
            wxo = singles.tile([C, 4 * H], f16)
            wh = singles.tile([128, KT, 4 * H], f16)
            wg = singles.tile([128, KT, C], f16)
            ident = singles.tile([128, 128], f16)
            cT = singles.tile([BC, H], f16)

            # ---- load everything ----
            for k in range(KT):
                nc.sync.dma_start(
                    out=h_tt2[:, :, k, :],
                    in_=h_t_d[k * 128:(k + 1) * 128])
            nc.sync.dma_start(out=oneh, in_=oneh_d[:])
            nc.sync.dma_start(
                out=wi2h, in_=wi2h_d[:].rearrange("(k p) h -> p k h", p=128))
            nc.sync.dma_start(
                out=wh2h, in_=wh2h_d[:].rearrange("(k p) h -> p k h", p=128))
            nc.sync.dma_start(
                out=bh2hT, in_=bh2h_d[:].rearrange("(k p) o -> p k o", p=128))
            nc.sync.dma_start(
                out=wsc, in_=wsc_d[:].rearrange("(k p) o -> p k o", p=128))
            nc.sync.dma_start(
                out=wxc, in_=wxc_d[:].rearrange("(k p) g -> p k g", p=128))
            nc.sync.dma_start(out=wxo, in_=wxo_d[:])
            nc.sync.dma_start(
                out=wh, in_=wh_d[:].rearrange("(k p) g -> p k g", p=128))
            nc.sync.dma_start(
                out=wg, in_=wg_d[:].rearrange("(k p) c -> p k c", p=128))
            make_identity(nc, ident)
            nc.vector.memset(hidT[:, :, 0, :], 0.0)
            nc.vector.memset(cT, 0.0)

            # ---- precompute projT2 = (batch_H @ Wi2h)^T, t-major ----
            TCH = NCHUNK // BC  # t columns per 512-chunk
            ci = 0
            for m in range(KT):
                for c in range(T // TCH):
                    ps = psE.tile([128, 2, NCHUNK], f32, tag="e")
                    for k in range(KT):
                        hs = h_tt2[:, c * TCH:(c + 1) * TCH, k, :]
                        nc.tensor.matmul(
                            ps[:, 0, :], wi2h[:, k, m * 128:(m + 1) * 128],
                            hs, start=(k == 0), stop=(k == KT - 1))
                    dst = projT2[:, m, c * TCH:(c + 1) * TCH, :]
                    if ci % 2 == 0:
                        nc.scalar.copy(
                            out=dst[:].rearrange("p t b -> p (t b)"),
                            in_=ps[:, 0, :])
                    else:
                        nc.vector.tensor_copy(
                            dst[:].rearrange("p t b -> p (t b)"), ps[:, 0, :])
                    ci += 1

            work = ctx.enter_context(tc.tile_pool(name="work", bufs=2))
            small = ctx.enter_context(tc.tile_pool(name="small", bufs=2))

            # ---- decode steps ----
            for s in range(S):
                # hp^T = (h @ Wh2h)^T : [h', m, b] in PSUM
                ps_hp = psM.tile([128, KT, 128], f32, tag="m")
                for m in range(KT):
                    for k in range(KT):
                        nc.tensor.matmul(
                            ps_hp[:, m, :], wh2h[:, k, m * 128:(m + 1) * 128],
                            hidT[:, k, s, :], start=(k == 0), stop=(k == KT - 1))
                # gates partial (h and onehot terms; PE filler during z/tanh)
                ps_g = psG.tile([BC, 4 * H], f32, tag="g")
                for nchk in range(2):
                    nsl = slice(nchk * NCHUNK, (nchk + 1) * NCHUNK)
                    for k in range(KT):
                        nc.tensor.matmul(
                            ps_g[:, nsl], hidT[:, k, s, :], wh[:, k, nsl],
                            start=(k == 0), stop=False)
                    nc.tensor.matmul(
                        ps_g[:, nsl], oneh[:, s, :], wxo[:, nsl],
                        start=False, stop=False)
                # hpT = hp + bias (f16); broadcast over t happens inside the
                # z-add via a stride-0 middle dim (inner stays stride-1)
                hpT = small.tile([128, KT, 128], f16, tag="hpT")
                for m in range(KT):
                    nc.vector.tensor_scalar(
                        out=hpT[:, m, :], in0=ps_hp[:, m, :],
                        scalar1=bh2hT[:, m, :], scalar2=None, op0=ALU.add)

                # ctx numerator accumulates in PSUM over all t via identity
                # matmuls with contiguous moving slices; 2-way interleave
                # (even/odd t' in separate column halves) broken by explicit
                # memset + start=False
                ps_ctx = psM.tile([128, 2, KT * 128], f32, tag="m")
                nc.vector.memset(ps_ctx, 0.0)
                # zf8 accumulates Z partials folded to 8 t-planes: [p, 8, b]
                zf8 = small.tile([128, 8, BC], f16, tag="zf8")

                # 8 half-quarters (8 t' each == one psE tile): shortens the
                # z->tanh->e latency and lets ctx work start sooner
                ars = [None] * NH
                for hq in range(NH + 1):
                    if hq < NH:
                        tsl = slice(hq * TH, (hq + 1) * TH)
                        z = work.tile([128, KT, TH, BC], f16, tag="z")
                        hb = hpT[:]
                        nc.vector.tensor_tensor(
                            out=z,
                            in0=projT2[:, :, tsl, :],
                            in1=bass.AP(tensor=hb.tensor, offset=hb.offset,
                                        ap=[hb.ap[0], [128, KT], [0, TH],
                                            [1, BC]]),
                            op=ALU.add)
                        th = work.tile([128, KT, TH * BC], f16, tag="th")
                        nc.scalar.activation(
                            out=th[:].rearrange("p k n -> p (k n)"),
                            in_=z[:].rearrange("p k t b -> p (k t b)"),
                            func=AF.Tanh)
                        # e = w_score . tanh, replicated on all partitions
                        ps_e = psE.tile([128, 2, NCHUNK], f32, tag="e")
                        for c in range(2):
                            for k in range(KT):
                                nc.tensor.matmul(
                                    ps_e[:, c, :], wsc[:, k, :],
                                    th[:, k, c * NCHUNK:(c + 1) * NCHUNK],
                                    start=(k == 0), stop=(k == KT - 1))
                        ar = work.tile([128, TH, BC], f16, tag="ar")
                        nc.scalar.activation(
                            out=ar[:].rearrange("p t b -> p (t b)"),
                            in_=ps_e[:].rearrange("p c n -> p (c n)"),
                            func=AF.Exp)
                        ars[hq] = ar
                    if hq >= 1:
                        # staggered ctx work for half-quarter hq-1
                        ar0 = ars[hq - 1]
                        tsl0 = slice((hq - 1) * TH, hq * TH)
                        a0 = ar0[:]
                        tmp = work.tile([128, TH, KT, BC], f16, tag="tmp")
                        nc.vector.tensor_tensor(
                            out=tmp, in0=h_tt2[:, tsl0, :, :],
                            in1=bass.AP(tensor=a0.tensor, offset=a0.offset,
                                        ap=[a0.ap[0], [BC, TH], [0, KT],
                                            [1, BC]]),
                            op=ALU.mult)
                        # PE reduce: t-pair moving slices (N=512)
                        for u in range(TH // 2):
                            nc.tensor.matmul(
                                ps_ctx[:].rearrange("p c n -> p (c n)"),
                                ident,
                                tmp[:, 2 * u:2 * u + 2, :, :].rearrange(
                                    "p t k b -> p (t k b)"),
                                start=False,
                                stop=(hq == NH and u == TH // 2 - 1),
                                skip_group_check=True)
                        # Z-fold once per completed quarter: pair the two
                        # half-tiles elementwise (t 16 -> 8). gpsimd early,
                        # DVE for the last quarter (it sits on the tail)
                        if (hq - 1) % 2 == 1:
                            qq = (hq - 1) // 2
                            ar_a = ars[hq - 2]
                            if qq == 0:
                                nc.gpsimd.tensor_tensor(
                                    out=zf8, in0=ar_a, in1=ar0, op=ALU.add)
                            else:
                                eng = (nc.vector if qq == NQ - 1
                                       else nc.gpsimd)
                                zq = small.tile([128, 8, BC], f16, tag="zq")
                                eng.tensor_tensor(
                                    out=zq, in0=ar_a, in1=ar0, op=ALU.add)
                                eng.tensor_tensor(
                                    out=zf8, in0=zf8, in1=zq, op=ALU.add)

                # Z: DVE folds zf8 [p, 8, b] down to [p, b]
                f3 = small.tile([128, 4, BC], f16, tag="zh3")
                nc.vector.tensor_tensor(out=f3, in0=zf8[:, 0:4],
                                        in1=zf8[:, 4:8], op=ALU.add)
                f4 = small.tile([128, 2, BC], f16, tag="zh4")
                nc.vector.tensor_tensor(out=f4, in0=f3[:, 0:2], in1=f3[:, 2:4],
                                        op=ALU.add)
                Zrep = small.tile([128, BC], f32, tag="Zrep")
                nc.vector.tensor_tensor(out=Zrep, in0=f4[:, 0], in1=f4[:, 1],
                                        op=ALU.add)
                rz = small.tile([128, BC], f32, tag="rz")
                nc.vector.reciprocal(out=rz, in_=Zrep)
                # ctxn = interleave-sum; ctxT = ctxn * rz (bcast over k)
                ctxn0 = small.tile([128, KT * 128], f32, tag="ctxn0")
                nc.scalar.copy(out=ctxn0, in_=ps_ctx[:, 0, :])
                ctxn = small.tile([128, KT * 128], f32, tag="ctxn")
                nc.vector.tensor_tensor(out=ctxn, in0=ctxn0,
                                        in1=ps_ctx[:, 1, :], op=ALU.add)
                ctxT = small.tile([128, KT, BC], f16, tag="ctxT")
                rzap = rz[:]
                nc.vector.tensor_tensor(
                    out=ctxT,
                    in0=ctxn[:].rearrange("p (k b) -> p k b", k=KT),
                    in1=bass.AP(tensor=rzap.tensor, offset=rzap.offset,
                                ap=[rzap.ap[0], [0, KT], [1, BC]]),
                    op=ALU.mult)

                # gates ctx part
                for nchk in range(2):
                    nsl = slice(nchk * NCHUNK, (nchk + 1) * NCHUNK)
                    for k in range(KT):
                        nc.tensor.matmul(
                            ps_g[:, nsl], ctxT[:, k, :], wxc[:, k, nsl],
                            start=False, stop=(k == KT - 1))

                # LSTM pointwise in [b, g] layout. keras order i, f, g, o
                # (b_lstm folded into wxo host-side).
                sig_if = small.tile([BC, 2 * H], f16, tag="sig_if")
                tg = small.tile([BC, H], f16, tag="tg")
                sig_o = small.tile([BC, H], f16, tag="sig_o")
                nc.scalar.activation(out=sig_if, in_=ps_g[:, 0:2 * H],
                                     func=AF.Sigmoid)
                nc.scalar.activation(out=tg, in_=ps_g[:, 2 * H:3 * H],
                                     func=AF.Tanh)
                nc.scalar.activation(out=sig_o, in_=ps_g[:, 3 * H:4 * H],
                                     func=AF.Sigmoid)
                t1 = small.tile([BC, H], f16, tag="t1")
                t2 = small.tile([BC, H], f16, tag="t2")
                nc.vector.tensor_tensor(out=t1, in0=sig_if[:, H:2 * H],
                                        in1=cT, op=ALU.mult)
                nc.vector.tensor_tensor(out=t2, in0=sig_if[:, 0:H], in1=tg,
                                        op=ALU.mult)
                nc.vector.tensor_tensor(out=cT, in0=t1, in1=t2, op=ALU.add)
                tc_t = small.tile([BC, H], f16, tag="tc_t")
                nc.scalar.activation(out=tc_t, in_=cT, func=AF.Tanh)
                h_bd = small.tile([BC, H], f16, tag="h_bd")
                nc.vector.tensor_tensor(out=h_bd, in0=sig_o, in1=tc_t,
                                        op=ALU.mult)
                # transpose h back to [h', k, b] via the DMA XBAR
                nc.sync.dma_start(
                    out=hidT[:, 0, s + 1, :],
                    in_=h_bd[:, 0:128], transpose=True)
                nc.scalar.dma_start(
                    out=hidT[:, 1, s + 1, :],
                    in_=h_bd[:, 128:256], transpose=True)

            # ---- generator: probsT = Wg^T . h_s for all steps; streamed out
            # via per-chunk DMAs (no big staging tile) ----
            hid_f = hidT[:].rearrange("p k s b -> p k (s b)")
            probs_df = probs_d[:].rearrange("c s b -> c (s b)")
            NS = S * BC  # 3328
            pos = 0
            ci = 0
            while pos < NS:
                n = min(NCHUNK, NS - pos)
                ps_p = psG.tile([C, NCHUNK], f32, tag="g")
                for k in range(KT):
                    nc.tensor.matmul(
                        ps_p[:, :n], wg[:, k, :], hid_f[:, k, BC + pos:BC + pos + n],
                        start=(k == 0), stop=(k == KT - 1))
                pchunk = small.tile([C, NCHUNK], f32, tag="pchunk")
                if ci % 2 == 0:
                    nc.scalar.copy(out=pchunk[:, :n], in_=ps_p[:, :n])
                else:
                    nc.vector.tensor_copy(pchunk[:, :n], ps_p[:, :n])
                nc.sync.dma_start(out=probs_df[:, pos:pos + n],
                                  in_=pchunk[:, :n])
                pos += n
                ci += 1

    _split_excess_waits(nc)
    return nc


def _get_module():
    if "nc" not in _CACHE:
        _CACHE["nc"] = _build()
    return _CACHE["nc"]


def build_in_maps(batch_H, text, batch_max_length, Wi2h, Wh2h, bh2h, w_score,
                  Wx, Wh, b_lstm, Wg, bg):
    batch_H = np.asarray(batch_H, dtype=np.float32)
    text = np.asarray(text)
    assert int(batch_max_length) + 1 == S
    assert batch_H.shape == (B, T, D)

    f16 = np.float16
    bh16 = batch_H.astype(f16)
    # one-hot text: [B, S, C] -> per-core [C, S, BC]
    oh = (text[:, :S, None] == np.arange(C)[None, None, :])

    Wx = np.asarray(Wx, np.float32)
    wxo_p = (Wx[D:D + C, :] + np.asarray(b_lstm, np.float32)[None, :]).astype(f16)
    weights = {
        "wi2h": np.ascontiguousarray(np.asarray(Wi2h, np.float32).astype(f16)),
        "wh2h": np.ascontiguousarray(np.asarray(Wh2h, np.float32).astype(f16)),
        "bh2hT": np.ascontiguousarray(
            np.asarray(bh2h, np.float32).reshape(H, 1)),
        "wsc": np.ascontiguousarray(np.tile(
            np.asarray(w_score, np.float32).reshape(H, 1), (1, 128)).astype(f16)),
        "wxc": np.ascontiguousarray(Wx[:D, :].astype(f16)),
        "wxo": np.ascontiguousarray(wxo_p),
        "wh": np.ascontiguousarray(np.asarray(Wh, np.float32).astype(f16)),
        "wg": np.ascontiguousarray(np.asarray(Wg, np.float32).astype(f16)),
    }

    in_maps = []
    for c in range(NCORES):
        bsl = slice(c * BC, (c + 1) * BC)
        in_maps.append({
            "h_t": np.ascontiguousarray(bh16[bsl].transpose(2, 1, 0)),
            "onehot": np.ascontiguousarray(
                oh[bsl].transpose(2, 1, 0).astype(f16)),
            **weights,
        })
    return in_maps


def kernel(**inputs):
    in_maps = build_in_maps(**inputs)
    bg = inputs["bg"]

    nc = _get_module()
    res = run_bass_kernel_spmd(nc, in_maps, list(range(NCORES)))

    out = np.empty((B, S, C), np.float32)
    for c in range(NCORES):
        out[c * BC:(c + 1) * BC] = res.results[c]["probsT"].transpose(2, 1, 0)
    out += np.asarray(bg, np.float32)[None, None, :]
    return out


if __name__ == "__main__":
    _build()
    print("build OK")


# revision 29
# speedup vs baseline: 1.1821x; 1.1821x over previous
"""Trainium2 Bass kernel for an attention-LSTM decoder (Bahdanau attention +
LSTM cell + generator head), data-parallel over 8 NeuronCores.

Shapes (hardcoded): B=1024, T=64, D=256, H=256, C=96, steps S=26.
Each core handles 128 batch rows.

Device layouts (per core, b = 128) — everything in the score/context chain
is stored T-MAJOR so every DVE op is stride-1-inner (2x mode) and the
context-reduce matmuls read contiguous moving slices:
  - h_tt2  [d' 128, t, k, b]   (H^T, t-major)
  - projT2 [h' 128, k, t, b]   (batch_H @ Wi2h)^T
  - z/th/ar/tmp per t-quarter, [p, k, t', b] / [p, t', k, b]
  - w_score replicated across 128 PE columns -> e (and exp(e)) come out of
    the e-matmul replicated on all partitions; context = h_tt2 * alphaR on
    DVE, reduced over t by PE identity-matmuls (contiguous moving slices,
    2-way PSUM interleave), already transposed [d', b] for the gates matmul.
  - Z = sum_t exp(e): gpsimd folds t' 16->8 per quarter; DVE folds the rest.
  - LSTM/gates run natural [b, 4H]; h transposed back via the DMA XBAR.
Matmul operands fp16, PSUM accumulation fp32.

Host-side prep (numpy): fp16 casts, batch_H transpose, one-hot text,
b_lstm folded into the one-hot weight rows, bg added on the host at the end.
"""

import sys

for _p in ("/opt/trn_rl_repo", "/root/.axon_site/_ro/trn_rl_repo"):
    if _p not in sys.path:
        sys.path.insert(0, _p)

import numpy as np

import concourse.bass as bass
import concourse.tile as tile
from concourse import mybir
from concourse.bass_utils import run_bass_kernel_spmd
from concourse.masks import make_identity

dt = mybir.dt
AF = mybir.ActivationFunctionType
ALU = mybir.AluOpType

NCORES = 8
B, T, D, H, C = 1024, 64, 256, 256, 96
S = 26  # num steps = batch_max_length + 1
BC = B // NCORES  # 128 batch rows per core
KT = 2  # 256 = 2 x 128 tiles for d/h contraction
TB = BC * T  # 8192 positions per step
NCHUNK = 512  # psum-bank-limited matmul N
NQ = 4  # t-quarters per step
TQ = T // NQ  # 16 t columns per quarter
NH = 8  # half-quarters
TH = T // NH  # 8 t columns per half-quarter

_CACHE = {}


def _split_excess_waits(nc, max_waits=1):
    """This container's walrus rejects instructions carrying more than
    ~max_waits semaphore waits ("Too many sync wait commands"). Hoist excess
    waits onto InstNoOp instructions inserted just before, on the same engine
    (per-engine program order makes this semantics-preserving)."""
    nid = [0]
    for f in nc.m.functions:
        for blk in f.blocks:
            insts = blk.instructions
            out = []
            changed = False
            for ins in insts:
                si = ins.sync_info
                ow = list(si.on_wait) if si is not None and si.on_wait else []
                if len(ow) > max_waits:
                    changed = True
                    while len(ow) > max_waits:
                        take, ow = ow[:max_waits], ow[max_waits:]
                        nid[0] += 1
                        nop = mybir.InstNoOp(
                            name=f"WSPLIT-{nid[0]}", engine=ins.engine,
                            sync_info=mybir.SyncInfo(on_wait=take,
                                                     on_update=[]))
                        nc.register_instruction(nop, overwrite=True)
                        out.append(nop)
                    ins.sync_info = mybir.SyncInfo(
                        on_wait=ow, on_update=list(si.on_update or []))
                out.append(ins)
            if changed:
                blk.instructions = out


def _build():
    nc = bass.Bass("TRN2", target_bir_lowering=False)
    f16, f32 = dt.float16, dt.float32

    h_t_d = nc.declare_dram_parameter("h_t", [D, T, BC], f16, isOutput=False)
    oneh_d = nc.declare_dram_parameter("onehot", [C, S, BC], f16, isOutput=False)
    wi2h_d = nc.declare_dram_parameter("wi2h", [D, H], f16, isOutput=False)
    wh2h_d = nc.declare_dram_parameter("wh2h", [H, H], f16, isOutput=False)
    bh2h_d = nc.declare_dram_parameter("bh2hT", [H, 1], f32, isOutput=False)
    wsc_d = nc.declare_dram_parameter("wsc", [H, 128], f16, isOutput=False)
    wxc_d = nc.declare_dram_parameter("wxc", [D, 4 * H], f16, isOutput=False)
    wxo_d = nc.declare_dram_parameter("wxo", [C, 4 * H], f16, isOutput=False)
    wh_d = nc.declare_dram_parameter("wh", [H, 4 * H], f16, isOutput=False)
    wg_d = nc.declare_dram_parameter("wg", [H, C], f16, isOutput=False)
    probs_d = nc.declare_dram_parameter("probsT", [C, S, BC], f32, isOutput=True)

    with tile.TileContext(nc) as tc:
        import contextlib
        ctx = contextlib.ExitStack()
        with ctx:
            singles = ctx.enter_context(tc.tile_pool(name="singles", bufs=1))
            psE = ctx.enter_context(tc.tile_pool(name="psE", bufs=2, space="PSUM"))
            psG = ctx.enter_context(tc.tile_pool(name="psG", bufs=1, space="PSUM"))
            psM = ctx.enter_context(tc.tile_pool(name="psM", bufs=2, space="PSUM"))

            # ---- persistent SBUF state ----
            h_tt2 = singles.tile([128, T, KT, BC], f16)  # H^T t-major
            projT2 = singles.tile([128, KT, T, BC], f16)
            hidT = singles.tile([128, KT, S + 1, BC], f16)  # h states; slot 0 = 0
            oneh = singles.tile([C, S, BC], f16)
            wi2h = singles.tile([128, KT, H], f16)
            wh2h = singles.tile([128, KT, H], f16)
            bh2hT = singles.tile([128, KT, 1], f32)
            wsc = singles.tile([128, KT, 128], f16)
            wxc = singles.tile([128, KT, 4 * H], f16)
            wxo = singles.tile([C, 4 * H], f16)
            wh = singles.tile([128, KT, 4 * H], f16)
            wg = singles.tile([128, KT, C], f16)
            ident = singles.tile([128, 128], f16)
            cT = singles.tile([BC, H], f16)

            # ---- load everything ----
            for k in range(KT):
                nc.sync.dma_start(
                    out=h_tt2[:, :, k, :],
                    in_=h_t_d[k * 128:(k + 1) * 128])
            nc.sync.dma_start(out=oneh, in_=oneh_d[:])
            nc.sync.dma_start(
                out=wi2h, in_=wi2h_d[:].rearrange("(k p) h -> p k h", p=128))
            nc.sync.dma_start(
                out=wh2h, in_=wh2h_d[:].rearrange("(k p) h -> p k h", p=128))
            nc.sync.dma_start(
                out=bh2hT, in_=bh2h_d[:].rearrange("(k p) o -> p k o", p=128))
            nc.sync.dma_start(
                out=wsc, in_=wsc_d[:].rearrange("(k p) o -> p k o", p=128))
            nc.sync.dma_start(
                out=wxc, in_=wxc_d[:].rearrange("(k p) g -> p k g", p=128))
            nc.sync.dma_start(out=wxo, in_=wxo_d[:])
            nc.sync.dma_start(
                out=wh, in_=wh_d[:].rearrange("(k p) g -> p k g", p=128))
            nc.sync.dma_start(
                out=wg, in_=wg_d[:].rearrange("(k p) c -> p k c", p=128))
            make_identity(nc, ident)
            nc.vector.memset(hidT[:, :, 0, :], 0.0)
            nc.vector.memset(cT, 0.0)

            # ---- precompute projT2 = (batch_H @ Wi2h)^T, t-major ----
            TCH = NCHUNK // BC  # t columns per 512-chunk
            ci = 0
            for m in range(KT):
                for c in range(T // TCH):
                    ps = psE.tile([128, 2, NCHUNK], f32, tag="e")
                    for k in range(KT):
                        hs = h_tt2[:, c * TCH:(c + 1) * TCH, k, :]
                        nc.tensor.matmul(
                            ps[:, 0, :], wi2h[:, k, m * 128:(m + 1) * 128],
                            hs, start=(k == 0), stop=(k == KT - 1))
                    dst = projT2[:, m, c * TCH:(c + 1) * TCH, :]
                    if ci % 2 == 0:
                        nc.scalar.copy(
                            out=dst[:].rearrange("p t b -> p (t b)"),
                            in_=ps[:, 0, :])
                    else:
                        nc.vector.tensor_copy(
                            dst[:].rearrange("p t b -> p (t b)"), ps[:, 0, :])
                    ci += 1

            work = ctx.enter_context(tc.tile_pool(name="work", bufs=2))
            arp = ctx.enter_context(tc.tile_pool(name="arp", bufs=3))
            small = ctx.enter_context(tc.tile_pool(name="small", bufs=2))

            # ---- decode steps ----
            for s in range(S):
                # hp^T = (h @ Wh2h)^T : [h', m, b] in PSUM
                ps_hp = psM.tile([128, KT, 128], f32, tag="m")
                for m in range(KT):
                    for k in range(KT):
                        nc.tensor.matmul(
                            ps_hp[:, m, :], wh2h[:, k, m * 128:(m + 1) * 128],
                            hidT[:, k, s, :], start=(k == 0), stop=(k == KT - 1))
                # gates partial (h and onehot terms; PE filler during z/tanh)
                ps_g = psG.tile([BC, 4 * H], f32, tag="g")
                for nchk in range(2):
                    nsl = slice(nchk * NCHUNK, (nchk + 1) * NCHUNK)
                    for k in range(KT):
                        nc.tensor.matmul(
                            ps_g[:, nsl], hidT[:, k, s, :], wh[:, k, nsl],
                            start=(k == 0), stop=False)
                    nc.tensor.matmul(
                        ps_g[:, nsl], oneh[:, s, :], wxo[:, nsl],
                        start=False, stop=False)
                # hpT = hp + bias (f16); broadcast over t happens inside the
                # z-add via a stride-0 middle dim (inner stays stride-1)
                hpT = small.tile([128, KT, 128], f16, tag="hpT")
                for m in range(KT):
                    nc.vector.tensor_scalar(
                        out=hpT[:, m, :], in0=ps_hp[:, m, :],
                        scalar1=bh2hT[:, m, :], scalar2=None, op0=ALU.add)

                # ctx numerator accumulates in PSUM over all t via identity
                # matmuls with contiguous moving slices; 2-way interleave
                # (even/odd t' in separate column halves) broken by explicit
                # memset + start=False
                ps_ctx = psM.tile([128, 2, KT * 128], f32, tag="m")
                nc.vector.memset(ps_ctx, 0.0)
                # zf8 accumulates Z partials folded to 8 t-planes: [p, 8, b]
                zf8 = small.tile([128, 8, BC], f16, tag="zf8")

                # 8 half-quarters (8 t' each == one psE tile): shortens the
                # z->tanh->e latency and lets ctx work start sooner
                ars = [None] * NH
                for hq in range(NH + 1):
                    if hq < NH:
                        tsl = slice(hq * TH, (hq + 1) * TH)
                        z = work.tile([128, KT, TH, BC], f16, tag="z")
                        hb = hpT[:]
                        nc.vector.tensor_tensor(
                            out=z,
                            in0=projT2[:, :, tsl, :],
                            in1=bass.AP(tensor=hb.tensor, offset=hb.offset,
                                        ap=[hb.ap[0], [128, KT], [0, TH],
                                            [1, BC]]),
                            op=ALU.add)
                        th = work.tile([128, KT, TH * BC], f16, tag="th")
                        nc.scalar.activation(
                            out=th[:].rearrange("p k n -> p (k n)"),
                            in_=z[:].rearrange("p k t b -> p (k t b)"),
                            func=AF.Tanh)
                        # e = w_score . tanh, replicated on all partitions
                        ps_e = psE.tile([128, 2, NCHUNK], f32, tag="e")
                        for c in range(2):
                            for k in range(KT):
                                nc.tensor.matmul(
                                    ps_e[:, c, :], wsc[:, k, :],
                                    th[:, k, c * NCHUNK:(c + 1) * NCHUNK],
                                    start=(k == 0), stop=(k == KT - 1))
                        ar = arp.tile([128, TH, BC], f16, tag="ar")
                        nc.scalar.activation(
                            out=ar[:].rearrange("p t b -> p (t b)"),
                            in_=ps_e[:].rearrange("p c n -> p (c n)"),
                            func=AF.Exp)
                        ars[hq] = ar
                    if hq >= 1:
                        # staggered ctx work for half-quarter hq-1
                        ar0 = ars[hq - 1]
                        tsl0 = slice((hq - 1) * TH, hq * TH)
                        a0 = ar0[:]
                        tmp = work.tile([128, TH, KT, BC], f16, tag="tmp")
                        nc.vector.tensor_tensor(
                            out=tmp, in0=h_tt2[:, tsl0, :, :],
                            in1=bass.AP(tensor=a0.tensor, offset=a0.offset,
                                        ap=[a0.ap[0], [BC, TH], [0, KT],
                                            [1, BC]]),
                            op=ALU.mult)
                        # PE reduce: t-pair moving slices (N=512)
                        for u in range(TH // 2):
                            nc.tensor.matmul(
                                ps_ctx[:].rearrange("p c n -> p (c n)"),
                                ident,
                                tmp[:, 2 * u:2 * u + 2, :, :].rearrange(
                                    "p t k b -> p (t k b)"),
                                start=False,
                                stop=(hq == NH and u == TH // 2 - 1),
                                skip_group_check=True)
                        # Z-fold once per completed quarter: pair the two
                        # half-tiles elementwise (t 16 -> 8). gpsimd early,
                        # DVE for the last quarter (it sits on the tail)
                        if (hq - 1) % 2 == 1:
                            qq = (hq - 1) // 2
                            ar_a = ars[hq - 2]
                            if qq == 0:
                                nc.gpsimd.tensor_tensor(
                                    out=zf8, in0=ar_a, in1=ar0, op=ALU.add)
                            else:
                                eng = (nc.vector if qq == NQ - 1
                                       else nc.gpsimd)
                                zq = small.tile([128, 8, BC], f16, tag="zq")
                                eng.tensor_tensor(
                                    out=zq, in0=ar_a, in1=ar0, op=ALU.add)
                                eng.tensor_tensor(
                                    out=zf8, in0=zf8, in1=zq, op=ALU.add)

                # Z: DVE folds zf8 [p, 8, b] down to [p, b]
                f3 = small.tile([128, 4, BC], f16, tag="zh3")
                nc.vector.tensor_tensor(out=f3, in0=zf8[:, 0:4],
                                        in1=zf8[:, 4:8], op=ALU.add)
                f4 = small.tile([128, 2, BC], f16, tag="zh4")
                nc.vector.tensor_tensor(out=f4, in0=f3[:, 0:2], in1=f3[:, 2:4],
                                        op=ALU.add)
                Zrep = small.tile([128, BC], f32, tag="Zrep")
                nc.vector.tensor_tensor(out=Zrep, in0=f4[:, 0], in1=f4[:, 1],
                                        op=ALU.add)
                rz = small.tile([128, BC], f32, tag="rz")
                nc.vector.reciprocal(out=rz, in_=Zrep)
                # ctxn = interleave-sum; ctxT = ctxn * rz (bcast over k)
                ctxn0 = small.tile([128, KT * 128], f32, tag="ctxn0")
                nc.scalar.copy(out=ctxn0, in_=ps_ctx[:, 0, :])
                ctxn = small.tile([128, KT * 128], f32, tag="ctxn")
                nc.vector.tensor_tensor(out=ctxn, in0=ctxn0,
                                        in1=ps_ctx[:, 1, :], op=ALU.add)
                ctxT = small.tile([128, KT, BC], f16, tag="ctxT")
                rzap = rz[:]
                nc.vector.tensor_tensor(
                    out=ctxT,
                    in0=ctxn[:].rearrange("p (k b) -> p k b", k=KT),
                    in1=bass.AP(tensor=rzap.tensor, offset=rzap.offset,
                                ap=[rzap.ap[0], [0, KT], [1, BC]]),
                    op=ALU.mult)

                # gates ctx part
                for nchk in range(2):
                    nsl = slice(nchk * NCHUNK, (nchk + 1) * NCHUNK)
                    for k in range(KT):
                        nc.tensor.matmul(
                            ps_g[:, nsl], ctxT[:, k, :], wxc[:, k, nsl],
                            start=False, stop=(k == KT - 1))

                # LSTM pointwise in [b, g] layout. keras order i, f, g, o
                # (b_lstm folded into wxo host-side).
                sig_if = small.tile([BC, 2 * H], f16, tag="sig_if")
                tg = small.tile([BC, H], f16, tag="tg")
                sig_o = small.tile([BC, H], f16, tag="sig_o")
                nc.scalar.activation(out=sig_if, in_=ps_g[:, 0:2 * H],
                                     func=AF.Sigmoid)
                nc.scalar.activation(out=tg, in_=ps_g[:, 2 * H:3 * H],
                                     func=AF.Tanh)
                nc.scalar.activation(out=sig_o, in_=ps_g[:, 3 * H:4 * H],
                                     func=AF.Sigmoid)
                t1 = small.tile([BC, H], f16, tag="t1")
                t2 = small.tile([BC, H], f16, tag="t2")
                nc.vector.tensor_tensor(out=t1, in0=sig_if[:, H:2 * H],
                                        in1=cT, op=ALU.mult)
                nc.vector.tensor_tensor(out=t2, in0=sig_if[:, 0:H], in1=tg,
                                        op=ALU.mult)
                nc.vector.tensor_tensor(out=cT, in0=t1, in1=t2, op=ALU.add)
                tc_t = small.tile([BC, H], f16, tag="tc_t")
                nc.scalar.activation(out=tc_t, in_=cT, func=AF.Tanh)
                h_bd = small.tile([BC, H], f16, tag="h_bd")
                nc.vector.tensor_tensor(out=h_bd, in0=sig_o, in1=tc_t,
                                        op=ALU.mult)
                # transpose h back to [h', k, b] via the DMA XBAR
                nc.sync.dma_start(
                    out=hidT[:, 0, s + 1, :],
                    in_=h_bd[:, 0:128], transpose=True)
                nc.scalar.dma_start(
                    out=hidT[:, 1, s + 1, :],
                    in_=h_bd[:, 128:256], transpose=True)

            # ---- generator: probsT = Wg^T . h_s for all steps; streamed out
            # via per-chunk DMAs (no big staging tile) ----
            hid_f = hidT[:].rearrange("p k s b -> p k (s b)")
            probs_df = probs_d[:].rearrange("c s b -> c (s b)")
            NS = S * BC  # 3328
            pos = 0
            ci = 0
            while pos < NS:
                n = min(NCHUNK, NS - pos)
                ps_p = psG.tile([C, NCHUNK], f32, tag="g")
                for k in range(KT):
                    nc.tensor.matmul(
                        ps_p[:, :n], wg[:, k, :], hid_f[:, k, BC + pos:BC + pos + n],
                        start=(k == 0), stop=(k == KT - 1))
                pchunk = small.tile([C, NCHUNK], f32, tag="pchunk")
                if ci % 2 == 0:
                    nc.scalar.copy(out=pchunk[:, :n], in_=ps_p[:, :n])
                else:
                    nc.vector.tensor_copy(pchunk[:, :n], ps_p[:, :n])
                nc.sync.dma_start(out=probs_df[:, pos:pos + n],
                                  in_=pchunk[:, :n])
                pos += n
                ci += 1

    _split_excess_waits(nc)
    return nc


def _get_module():
    if "nc" not in _CACHE:
        _CACHE["nc"] = _build()
    return _CACHE["nc"]


def build_in_maps(batch_H, text, batch_max_length, Wi2h, Wh2h, bh2h, w_score,
                  Wx, Wh, b_lstm, Wg, bg):
    batch_H = np.asarray(batch_H, dtype=np.float32)
    text = np.asarray(text)
    assert int(batch_max_length) + 1 == S
    assert batch_H.shape == (B, T, D)

    f16 = np.float16
    bh16 = batch_H.astype(f16)
    # one-hot text: [B, S, C] -> per-core [C, S, BC]
    oh = (text[:, :S, None] == np.arange(C)[None, None, :])

    Wx = np.asarray(Wx, np.float32)
    wxo_p = (Wx[D:D + C, :] + np.asarray(b_lstm, np.float32)[None, :]).astype(f16)
    weights = {
        "wi2h": np.ascontiguousarray(np.asarray(Wi2h, np.float32).astype(f16)),
        "wh2h": np.ascontiguousarray(np.asarray(Wh2h, np.float32).astype(f16)),
        "bh2hT": np.ascontiguousarray(
            np.asarray(bh2h, np.float32).reshape(H, 1)),
        "wsc": np.ascontiguousarray(np.tile(
            np.asarray(w_score, np.float32).reshape(H, 1), (1, 128)).astype(f16)),
        "wxc": np.ascontiguousarray(Wx[:D, :].astype(f16)),
        "wxo": np.ascontiguousarray(wxo_p),
        "wh": np.ascontiguousarray(np.asarray(Wh, np.float32).astype(f16)),
        "wg": np.ascontiguousarray(np.asarray(Wg, np.float32).astype(f16)),
    }

    in_maps = []
    for c in range(NCORES):
        bsl = slice(c * BC, (c + 1) * BC)
        in_maps.append({
            "h_t": np.ascontiguousarray(bh16[bsl].transpose(2, 1, 0)),
            "onehot": np.ascontiguousarray(
                oh[bsl].transpose(2, 1, 0).astype(f16)),
            **weights,
        })
    return in_maps


def kernel(**inputs):
    in_maps = build_in_maps(**inputs)
    bg = inputs["bg"]

    nc = _get_module()
    res = run_bass_kernel_spmd(nc, in_maps, list(range(NCORES)))

    out = np.empty((B, S, C), np.float32)
    for c in range(NCORES):
        out[c * BC:(c + 1) * BC] = res.results[c]["probsT"].transpose(2, 1, 0)
    out += np.asarray(bg, np.float32)[None, None, :]
    return out


if __name__ == "__main__":
    _build()
    print("build OK")
